# revision 2
# baseline (speedup 1.0000x reference)
"""Bahdanau attention TRN2 Bass kernel, v2.

Same contract as kernel.py: kernel(**inputs) takes FULL inputs, returns
(ctx [32,2048] f32, attn [32,2048] f32). Data-parallel over batch, 4
batch elements per core on 8 cores.

v2 redesign vs baseline:
  - enc is loaded ONCE per chunk in e-major layout (encT) only.  The
    l-major copy (encN) is gone: the context reduction now contracts
    along the FREE dim of encT using the Pool engine's
    scalar_tensor_tensor with accum_out (per-partition mult+reduce),
    which yields ctx transposed ([128, ET] with e = t*128+p).  DMA
    traffic halves (32MB/core instead of 64MB).
  - scores are reduced with a ones MATRIX stationary, broadcasting the
    chunk's scores to all 128 partitions in one matmul.  exp of that
    broadcast gives the per-partition weight rows the ctx reduction
    needs directly -- the per-chunk exp -> SBUF-SBUF DMA -> PE
    transpose chain of the baseline (which put ~2.7us of DMA latency on
    the PE critical path every chunk) is gone.
  - ctx accumulation lives entirely on the (otherwise idle) Pool
    engine; DVE keeps only the v-dot FMA chain and small bookkeeping.
  - PE tail work for chunk i (scores reduce) and batch outputs are
    emitted AFTER the first j-blocks of chunk i+1's projection matmuls,
    so the in-order PE queue never waits on cross-engine tails.
  - whT is stored j-major (whJ) and loaded in 8 per-j slabs so the
    first projection chains can start before the whole 4MB arrives.
"""

import json as _json
from contextlib import ExitStack

import numpy as np
import ml_dtypes

_BF16 = ml_dtypes.bfloat16

_B, _L, _D, _E = 32, 2048, 1024, 2048
_NCORES = 8
_P = 128
_LCHUNK = 512


# ----------------------------------------------------------------------------
# Workaround: this walrus build rejects instructions carrying more than one
# semaphore wait ("Too many sync wait commands").  Split extra waits onto
# preceding same-engine NoOps at BIR-serialization time.
# ----------------------------------------------------------------------------
_ws_counter = [0]


def _split_instruction_waits(inst, max_waits=1):
    waits = inst.get("sync_info", {}).get("on_wait") or []
    if len(waits) <= max_waits:
        return [inst]
    out = []
    extra = waits[:-max_waits]
    inst["sync_info"]["on_wait"] = waits[-max_waits:]
    for i in range(0, len(extra), max_waits):
        _ws_counter[0] += 1
        out.append({
            "debug": inst.get("debug", 0),
            "engine": inst["engine"],
            "ins": [],
            "name": f"I-ws{_ws_counter[0]}",
            "opcode": "NoOp",
            "outs": [],
            "sync_info": {"on_update": [], "on_wait": extra[i:i + max_waits]},
        })
    out.append(inst)
    return out


def _walk_split(obj):
    if isinstance(obj, dict):
        for key, val in obj.items():
            if key == "instructions" and isinstance(val, list):
                new = []
                for inst in val:
                    if isinstance(inst, dict) and "sync_info" in inst:
                        new.extend(_split_instruction_waits(inst))
                    else:
                        _walk_split(inst)
                        new.append(inst)
                obj[key] = new
            else:
                _walk_split(val)
    elif isinstance(obj, list):
        for item in obj:
            _walk_split(item)


def _install_waitsplit():
    import concourse.bass as bass
    if getattr(bass.Bass, "_waitsplit_installed", False):
        return
    orig = bass.Bass.to_json_bytes

    def to_json_bytes(self, *a, **kw):
        d = _json.loads(orig(self, *a, **kw))
        _walk_split(d)
        return _json.dumps(d).encode()

    bass.Bass.to_json_bytes = to_json_bytes
    bass.Bass._waitsplit_installed = True


# ----------------------------------------------------------------------------
# Kernel builder
# ----------------------------------------------------------------------------

def _build(Bc=4, L=_L, D=_D, E=_E, LCHUNK=_LCHUNK, reps=1):
    import concourse.bass as bass
    import concourse.mybir as mybir
    import concourse.tile as tile
    from concourse.masks import make_identity

    F32 = mybir.dt.float32
    F32R = mybir.dt.float32r
    BF16 = mybir.dt.bfloat16
    AF = mybir.ActivationFunctionType
    ALU = mybir.AluOpType

    P = _P
    DT, ET = D // P, E // P          # 8, 16
    NCH = L // LCHUNK                # 4
    assert D % P == 0 and E % P == 0 and L % LCHUNK == 0

    nc = bass.Bass("TRN2", target_bir_lowering=False, debug=False)
    encT = nc.dram_tensor("encT", [Bc, NCH, P, ET * LCHUNK], BF16,
                          kind="ExternalInput").ap()
    # j-major W_h^T: whJ[p, j, t, d] = W_h[j*128+d, t*128+p]
    whJ = nc.dram_tensor("whJ", [P, DT * ET * P], BF16,
                         kind="ExternalInput").ap()
    wsT = nc.dram_tensor("wsT", [P, DT * D], BF16, kind="ExternalInput").ap()
    decT = nc.dram_tensor("decT", [P, DT * Bc], BF16,
                          kind="ExternalInput").ap()
    vT = nc.dram_tensor("vT", [P, DT], BF16, kind="ExternalInput").ap()
    ctx_o = nc.dram_tensor("ctx", [Bc, ET, P], F32, kind="ExternalOutput").ap()
    attn_o = nc.dram_tensor("attn", [Bc, L], F32, kind="ExternalOutput").ap()

    with tile.TileContext(nc) as tc:
        with ExitStack() as es:
            const_p = es.enter_context(tc.tile_pool(name="const", bufs=1))
            w_p = es.enter_context(tc.tile_pool(name="wp", bufs=1))
            encT_p = es.enter_context(tc.tile_pool(name="encTp", bufs=4))
            tj_p = es.enter_context(tc.tile_pool(name="tjp", bufs=4))
            u_p = es.enter_context(tc.tile_pool(name="up", bufs=2))
            eb_p = es.enter_context(tc.tile_pool(name="ebp", bufs=2))
            junk_p = es.enter_context(tc.tile_pool(name="junkp", bufs=4))
            rows_p = es.enter_context(tc.tile_pool(name="rows", bufs=1))

            # constants
            ident = const_p.tile([P, P], F32)
            make_identity(nc, ident[:])
            ones_raw = const_p.tile([P, P], F32, tag="ones_raw",
                                    name="ones_raw")
            nc.vector.memset(ones_raw[:], 1.0)
            ones_m = const_p.tile([P, P], F32, tag="ones_m", name="ones_m")
            nc.vector.tensor_copy(out=ones_m[:].bitcast(F32R),
                                  in_=ones_raw[:])

            state = {}

            def emit_load(b, c):
                eT = encT_p.tile([P, ET * LCHUNK], BF16, tag="encT",
                                 name=f"encT{b}_{c}")
                nc.sync.dma_start(eT[:], encT[b, c])
                state[(b, c)] = eT

            # first two chunks on the sync ring immediately
            emit_load(0, 0)
            emit_load(0, 1)

            # ---- s inputs first on the ACT ring (s-proj gates tanh j0),
            # then the whJ slabs in j order (slab j gates PE chain j).
            wst_sb = w_p.tile([P, DT * D], BF16, tag="wst", name="wst_sb")
            nc.scalar.dma_start(wst_sb[:], wsT[:, :])
            dec_sb = w_p.tile([P, DT * Bc], BF16, tag="dec_sb", name="dec_sb")
            nc.scalar.dma_start(dec_sb[:], decT[:, :])
            v_sb = w_p.tile([P, DT], BF16, tag="v_sb", name="v_sb")
            nc.scalar.dma_start(v_sb[:], vT[:, :])
            v32 = w_p.tile([P, DT], F32, tag="v32", name="v32")
            nc.vector.tensor_copy(out=v32[:], in_=v_sb[:])
            whJ_sb = w_p.tile([P, DT * ET * P], BF16, tag="whJ",
                              name="whJ_sb")
            JSL = ET * P
            for j in range(DT):
                nc.scalar.dma_start(whJ_sb[:, j * JSL:(j + 1) * JSL],
                                    whJ[:, j * JSL:(j + 1) * JSL])

            # ---- s-projection: sT[:, j*Bc + b] = s_b[j*128 + p] ----
            # NOTE: start=True clears has_written for the WHOLE PSUM bank, so
            # each of the 8 accumulation chains needs its own bank.  The
            # setup pool closes before the main PSUM pools open.
            sT = w_p.tile([P, DT * Bc], F32, tag="sT", name="sT_sb")
            with tc.tile_pool(name="ps_setup", bufs=1, space="PSUM") as ps_set:
                ps_s = [ps_set.tile([P, Bc], F32, tag=f"s{J}", name=f"ps_s{J}")
                        for J in range(DT)]
                for t in range(DT):
                    for J in range(DT):
                        nc.tensor.matmul(
                            ps_s[J][:],
                            wst_sb[:, t * D + J * P:t * D + (J + 1) * P],
                            dec_sb[:, t * Bc:(t + 1) * Bc],
                            start=(t == 0), stop=(t == DT - 1))
                for J in range(DT):
                    nc.vector.tensor_copy(out=sT[:, J * Bc:(J + 1) * Bc],
                                          in_=ps_s[J][:])

            ps_h = es.enter_context(tc.tile_pool(name="ps_h", bufs=4,
                                                 space="PSUM"))
            ps_b = es.enter_context(tc.tile_pool(name="ps_b", bufs=2,
                                                 space="PSUM"))
            ps_x = es.enter_context(tc.tile_pool(name="ps_x", bufs=1,
                                                 space="PSUM"))

            def emit_jblock(b, c, j):
                """PE projection chain j of chunk (b,c) + tanh + v-FMA."""
                eT = state[(b, c)]
                ph = ps_h.tile([P, LCHUNK], F32, tag="ph", name="ph")
                for t in range(ET):
                    nc.tensor.matmul(
                        ph[:],
                        whJ_sb[:, j * JSL + t * P:j * JSL + (t + 1) * P],
                        eT[:, t * LCHUNK:(t + 1) * LCHUNK],
                        start=(t == 0), stop=(t == ET - 1))
                tj = tj_p.tile([P, LCHUNK], F32, tag="tj", name="tj")
                nc.scalar.activation(tj[:], ph[:], AF.Tanh,
                                     bias=sT[:, j * Bc + b:j * Bc + b + 1])
                U = state[("U", b, c)]
                if j == 0:
                    nc.vector.tensor_scalar_mul(U[:].bitcast(F32R), tj[:],
                                                v32[:, 0:1])
                else:
                    nc.vector.scalar_tensor_tensor(
                        U[:].bitcast(F32R), tj[:], v32[:, j:j + 1], U[:],
                        ALU.mult, ALU.add)

            def emit_tail(b, c):
                """Scores reduce + exp broadcast + ctx reduction + (for the
                last chunk) batch outputs.  Deferred: runs after chunk
                (b,c+1)'s first j-blocks so the PE never waits on it."""
                eT = state.pop((b, c))
                U = state.pop(("U", b, c))
                # broadcast scores to all partitions: B[m, l] = scores[l]
                Bt = ps_b.tile([P, LCHUNK], F32, tag="B", name="B")
                nc.tensor.matmul(Bt[:], ones_m[:].bitcast(F32R),
                                 U[:].bitcast(F32R), start=True, stop=True)
                EB = eb_p.tile([P, LCHUNK], BF16, tag="EB", name="EB")
                zc = rows_p.tile([P, 1], F32, tag=f"zc{c % 2}", name="zc")
                # exp without max-subtraction: |scores| <= sum|v| ~ 26.
                # EB is the bf16 broadcast weight block for the ctx
                # reduction; the f32 attn row is written by a second, cheap
                # single-partition exp directly from the PSUM scores.
                nc.scalar.activation(EB[:], Bt[:], AF.Exp, accum_out=zc[:])
                if c == 0:
                    erow = rows_p.tile([1, L], F32, tag=f"erow{b % 2}",
                                       name=f"erow{b}")
                    state[("erow", b)] = erow
                    zb = rows_p.tile([P, 1], F32, tag=f"zb{b % 2}", name="zb")
                    nc.vector.tensor_copy(out=zb[:], in_=zc[:])
                    state[("zb", b)] = zb
                else:
                    erow = state[("erow", b)]
                    zb = state[("zb", b)]
                    nc.vector.tensor_add(out=zb[:], in0=zb[:], in1=zc[:])
                nc.scalar.activation(erow[:, c * LCHUNK:(c + 1) * LCHUNK],
                                     Bt[0:1, :], AF.Exp)

                # ctx reduction on DVE: ctxc[p, t] = sum_l eT[p, t*512+l] * w[l]
                # (walrus rejects tensor ops on Pool and can't codegen
                # InstTensorTensorReduce, so: native TensorScalarPtr with
                # accum_out; all-bf16 operands enable the 2x DVE mode)
                ctxc = rows_p.tile([P, ET], F32, tag=f"ctxc{c % 2}",
                                   name="ctxc")
                for t in range(ET):
                    junk = junk_p.tile([P, LCHUNK], BF16, tag="junk",
                                       name="junk")
                    nc.vector.scalar_tensor_tensor(
                        junk[:], eT[:, t * LCHUNK:(t + 1) * LCHUNK], 1.0,
                        EB[:], ALU.mult, ALU.mult,
                        accum_out=ctxc[:, t:t + 1])
                if c == 0:
                    ctxA = rows_p.tile([P, ET], F32, tag=f"ctxA{b % 2}",
                                       name=f"ctxA{b}")
                    nc.vector.tensor_copy(out=ctxA[:], in_=ctxc[:])
                    state[("ctxA", b)] = ctxA
                else:
                    ctxA = state[("ctxA", b)]
                    nc.vector.tensor_add(out=ctxA[:], in0=ctxA[:],
                                         in1=ctxc[:])

                if c == NCH - 1:
                    # defer one more flush cycle: by then the Pool/DVE chain
                    # for this batch has long finished and the PE transpose
                    # issues without waiting.
                    pending.append(lambda: emit_batch_out(b))

            def emit_batch_out(b):
                erow = state.pop(("erow", b))
                zb = state.pop(("zb", b))
                ctxA = state.pop(("ctxA", b))
                rz = rows_p.tile([P, 1], F32, tag=f"rz{b % 2}", name="rz")
                nc.vector.reciprocal(rz[:], zb[:])
                nc.vector.tensor_scalar_mul(erow[:], erow[:], rz[0:1, 0:1])
                nc.scalar.dma_start(attn_o[b:b + 1, :], erow[:])
                nc.vector.tensor_scalar_mul(ctxA[:], ctxA[:], rz[:])
                # transpose [128, ET] -> [ET, 128] so the store is contiguous
                px = ps_x.tile([ET, P], F32, tag="px", name="px")
                nc.tensor.transpose(px[:], ctxA[:], ident[:])
                cxr = rows_p.tile([ET, P], F32, tag=f"cxr{b % 2}",
                                  name=f"cxr{b}")
                nc.vector.tensor_copy(out=cxr[:], in_=px[:])
                nc.scalar.dma_start(ctx_o[b], cxr[:])

            chunks = [(b, c) for b in range(Bc) for c in range(NCH)]
            pending = []

            def flush_pending():
                cur = pending[:]
                pending.clear()
                for fn in cur:
                    fn()

            for rep in range(reps):
                for i, (b, c) in enumerate(chunks):
                    state[("U", b, c)] = u_p.tile([P, LCHUNK], F32, tag="U",
                                                  name="U")
                    nxt = i + 2
                    if nxt < len(chunks):
                        emit_load(*chunks[nxt])
                    elif rep + 1 < reps:
                        emit_load(*chunks[nxt - len(chunks)])
                    for j in range(DT):
                        emit_jblock(b, c, j)
                        if j == 1:
                            flush_pending()
                    pending.append(lambda b=b, c=c: emit_tail(b, c))
            while pending:
                flush_pending()

    return nc


_cache = {}


def _get_nc(reps=1):
    key = ("nc", reps)
    if key not in _cache:
        _install_waitsplit()
        _cache[key] = _build(reps=reps)
    return _cache[key]


def prepare_in_maps(inputs):
    P = _P
    B, L, D, E = _B, _L, _D, _E
    LCHUNK = _LCHUNK
    DT, ET, NCH = D // P, E // P, L // LCHUNK
    Bc = B // _NCORES

    enc = np.asarray(inputs["enc_outputs"], dtype=np.float32)
    dec = np.asarray(inputs["dec_hidden"], dtype=np.float32)
    W_s = np.asarray(inputs["W_s"], dtype=np.float32)
    W_h = np.asarray(inputs["W_h"], dtype=np.float32)
    v = np.asarray(inputs["v"], dtype=np.float32)

    enc_bf = enc.astype(_BF16)
    # encT[b, c, p, t, l] = enc[b, c*LCHUNK + l, t*128 + p]
    encT = np.ascontiguousarray(
        enc_bf.reshape(B, NCH, LCHUNK, ET, P).transpose(0, 1, 4, 3, 2)
    ).reshape(B, NCH, P, ET * LCHUNK)
    # whJ[p, j, t, d] = W_h[j*128+d, t*128+p]
    whJ = np.ascontiguousarray(
        W_h.T.astype(_BF16).reshape(ET, P, DT, P).transpose(1, 2, 0, 3)
    ).reshape(P, DT * ET * P)
    # wsT[p, t, d] = W_s[d, t*128 + p]
    wsT = np.ascontiguousarray(
        W_s.T.astype(_BF16).reshape(DT, P, D).transpose(1, 0, 2)
    ).reshape(P, DT * D)
    vT = np.ascontiguousarray(v.astype(_BF16).reshape(DT, P).T)

    in_maps = []
    for i in range(_NCORES):
        dcore = dec[i * Bc:(i + 1) * Bc]
        # decT[p, t, b] = dec[b, t*128 + p]
        decT = np.ascontiguousarray(
            dcore.T.astype(_BF16).reshape(DT, P, Bc).transpose(1, 0, 2)
        ).reshape(P, DT * Bc)
        in_maps.append({
            "encT": encT[i * Bc:(i + 1) * Bc],
            "whJ": whJ,
            "wsT": wsT,
            "decT": decT,
            "vT": vT,
        })
    return in_maps


def run(inputs, trace=False, **run_kwargs):
    """Run on 8 NeuronCores; returns (ctx, attn, BassKernelResults)."""
    from concourse.bass_utils import run_bass_kernel_spmd

    nc = _get_nc()
    in_maps = prepare_in_maps(inputs)
    res = run_bass_kernel_spmd(nc, in_maps, core_ids=list(range(_NCORES)),
                               trace=trace, **run_kwargs)
    ctx = np.concatenate(
        [res.results[i]["ctx"].reshape(-1, _E) for i in range(_NCORES)],
        axis=0)
    attn = np.concatenate([res.results[i]["attn"] for i in range(_NCORES)],
                          axis=0)
    return ctx, attn, res


def kernel(**inputs):
    ctx, attn, _ = run(inputs, trace=False)
    return ctx, attn
